# revision 15
# baseline (speedup 1.0000x reference)
"""Trainium2 Bass kernel for a 3-layer bidirectional GRU classifier.

Model (faithful to the reference):
  BatchNorm1d (train-mode batch stats over (N,T)) -> 3 stacked biGRU layers
  (both "directions" scan forward in time, per the reference's quirk) ->
  last-timestep hidden concat -> Linear to 60 classes.

Strategy: pure data parallelism over batch (8 samples per core, 8 cores, no
collectives).  Per core the recurrent gate matmul streams W_hh^T through the
PE array with the per-sample hidden states h^T as the stationary operand,
using 4-way column tiling (tile_position col groups) so four weight-quarter
streams run concurrently.  The two directions ping-pong so DVE/ACT gate math
of one direction overlaps PE streaming of the other.  Input projections (gi)
for layer l+1 are computed as large matmuls from the transposed layer-l
outputs (built incrementally by per-step PE transposes) and staged in DRAM.
BatchNorm is folded into the layer-0 input projection on device.
"""

import os
import sys

for _p in ("/opt/trn_rl_repo", "/root/.axon_site/_ro/trn_rl_repo"):
    if os.path.isdir(_p) and _p not in sys.path:
        sys.path.append(_p)

import numpy as np
import ml_dtypes

import concourse.bacc as bacc
import concourse.mybir as mybir
from concourse.tile import TileContext
from concourse.bass_utils import run_bass_kernel_spmd

BF16 = mybir.dt.bfloat16
F32 = mybir.dt.float32
F32R = mybir.dt.float32r
AF = mybir.ActivationFunctionType
ALU = mybir.AluOpType

NCORES = 8
DEBUG_INIT = False  # memset garbage regions so CoreSim's uninit checks pass
NO_TRANSPOSE = False  # bisect flag: skip PE transposes (breaks correctness)
H = 1024
G3 = 3 * H
CIN = 150
NCLS = 60
EPS = 1e-5


def _perm3h(v):
    """Reorder a [..., 3072] gate-major vector to (g, a, cj) column order."""
    a = v.reshape(*v.shape[:-1], 3, 4, 256)
    a = np.moveaxis(a, -3, -2)  # [..., 4, 3, 256]
    return a.reshape(*v.shape[:-1], G3)


def host_prep(inputs, T, n_full):
    B = n_full // NCORES
    x = np.asarray(inputs["x"], np.float32)
    gamma = np.asarray(inputs["bn_gamma"], np.float32)
    beta = np.asarray(inputs["bn_beta"], np.float32)
    w_ih0 = np.asarray(inputs["w_ih0"], np.float32)
    w_hh0 = np.asarray(inputs["w_hh0"], np.float32)
    b_ih0 = np.asarray(inputs["b_ih0"], np.float32)
    b_hh0 = np.asarray(inputs["b_hh0"], np.float32)
    w_ih = np.asarray(inputs["w_ih"], np.float32)
    w_hh = np.asarray(inputs["w_hh"], np.float32)
    b_ih = np.asarray(inputs["b_ih"], np.float32)
    b_hh = np.asarray(inputs["b_hh"], np.float32)
    fc_w = np.asarray(inputs["fc_w"], np.float32)
    fc_b = np.asarray(inputs["fc_b"], np.float32)

    shared = {}
    NTF = T * n_full

    xTf = np.ascontiguousarray(x.transpose(2, 1, 0).reshape(CIN, NTF))
    shared["xtf0"] = np.ascontiguousarray(xTf[:128])
    xtf1 = np.zeros((32, NTF), np.float32)
    xtf1[: CIN - 128] = xTf[128:]
    shared["xtf1"] = xtf1

    # W_hh^T permuted bf16: [3, 2, 128, 8*3072], free = k*3072+g*768+a*256+cj
    whh_all = np.stack([w_hh0, w_hh[0], w_hh[1]])  # [3,2,3072,1024]
    t = whh_all.reshape(3, 2, 3, 4, 256, 8, 128)
    t = np.transpose(t, (0, 1, 6, 5, 3, 2, 4))
    shared["whh"] = np.ascontiguousarray(
        t.reshape(3, 2, 128, 8 * 3072).astype(ml_dtypes.bfloat16))

    # W_ih layers 1,2: [2, 16, 128, 6144], col = d*3072+g*768+a*256+cj
    t = w_ih.reshape(2, 2, 3, 4, 256, 16, 128)
    t = np.transpose(t, (0, 5, 6, 1, 3, 2, 4))
    shared["wih12"] = np.ascontiguousarray(
        t.reshape(2, 16, 128, 6144).astype(ml_dtypes.bfloat16))

    # W_ih0^T permuted fp32 [150->(128,32), 6144]
    t = w_ih0.reshape(2, 3, 4, 256, CIN)
    t = np.transpose(t, (4, 0, 2, 1, 3)).reshape(CIN, 6144)
    shared["wih0a"] = np.ascontiguousarray(t[:128])
    w0b = np.zeros((32, 6144), np.float32)
    w0b[: CIN - 128] = t[128:]
    shared["wih0b"] = w0b

    mask_rz = np.zeros(G3, np.float32)
    mask_rz[: 2 * H] = 1.0
    bias0 = np.concatenate(
        [_perm3h(b_ih0[d] + b_hh0[d] * mask_rz) for d in range(2)])
    shared["bias0"] = bias0.reshape(1, 6144)

    b12 = [np.concatenate(
        [_perm3h(b_ih[li, d] + b_hh[li, d] * mask_rz) for d in range(2)])
        for li in range(2)]
    shared["bias12"] = np.stack(b12).reshape(2, 1, 6144).astype(
        ml_dtypes.bfloat16)

    bhh_all = np.stack([b_hh0, b_hh[0], b_hh[1]])[:, :, 2 * H:]  # [3,2,1024]
    shared["bhhn"] = np.ascontiguousarray(
        bhh_all.reshape(3, 2, 1, 1024).astype(ml_dtypes.bfloat16))

    shared["fcwT"] = np.ascontiguousarray(
        fc_w.T.reshape(16, 128, NCLS).astype(ml_dtypes.bfloat16))
    shared["fcb"] = fc_b.reshape(1, NCLS).astype(ml_dtypes.bfloat16)

    shared["gamma"] = np.concatenate(
        [gamma, np.zeros(10, np.float32)]).reshape(160, 1)
    shared["beta"] = np.concatenate(
        [beta, np.zeros(10, np.float32)]).reshape(160, 1)

    shared["ones_bf"] = np.ones((1, 128), ml_dtypes.bfloat16)
    # identity rows replicated at each 32-partition block (for col-group
    # local transposes): row p holds e_{p%32} when p%32 < B else 0
    idr = np.zeros((128, B), ml_dtypes.bfloat16)
    for p in range(128):
        if p % 32 < B:
            idr[p, p % 32] = 1
    shared["identr"] = idr
    shared["ident128"] = np.eye(128, dtype=ml_dtypes.bfloat16)

    per_core = []
    for c in range(NCORES):
        m = dict(shared)
        xo = x[c * B: (c + 1) * B]
        xT = xo.transpose(2, 1, 0).reshape(CIN, T * B)
        aug = np.zeros((160, T * B), np.float32)
        aug[:CIN] = xT
        aug[CIN] = 1.0
        m["xto"] = aug.astype(ml_dtypes.bfloat16)
        per_core.append(m)
    return per_core


def build_program(T, n_full):
    B = n_full // NCORES
    ROWS = T * B
    NTF = T * n_full
    nc = bacc.Bacc("TRN2", target_bir_lowering=False, debug=False,
                   num_devices=NCORES)

    inp = {}
    def din(name, shape, dt):
        inp[name] = nc.dram_tensor(name, list(shape), dt, kind="ExternalInput")

    din("xtf0", (128, NTF), F32)
    din("xtf1", (32, NTF), F32)
    din("xto", (160, ROWS), BF16)
    din("whh", (3, 2, 128, 8 * 3072), BF16)
    din("wih12", (2, 16, 128, 6144), BF16)
    din("wih0a", (128, 6144), F32)
    din("wih0b", (32, 6144), F32)
    din("bias0", (1, 6144), F32)
    din("bias12", (2, 1, 6144), BF16)
    din("bhhn", (3, 2, 1, 1024), BF16)
    din("fcwT", (16, 128, NCLS), BF16)
    din("fcb", (1, NCLS), BF16)
    din("gamma", (160, 1), F32)
    din("beta", (160, 1), F32)
    din("ones_bf", (1, 128), BF16)
    din("identr", (128, B), BF16)
    din("ident128", (128, 128), BF16)

    out_t = nc.dram_tensor("out", [B, NCLS], F32, kind="ExternalOutput")

    WSTEPS = 64
    windows = []
    t0 = 0
    while t0 < T:
        windows.append((t0, min(t0 + WSTEPS, T)))
        t0 = min(t0 + WSTEPS, T)

    def mchunks(r0, r1):
        out = []
        while r0 < r1:
            out.append((r0, min(r0 + 128, r1)))
            r0 = min(r0 + 128, r1)
        return out

    with TileContext(nc) as tc:
        from contextlib import ExitStack
        ctx = ExitStack()
        pers = ctx.enter_context(tc.tile_pool(name="pers", bufs=1))
        gates_pool = ctx.enter_context(
            tc.tile_pool(name="gates", bufs=1, space="PSUM"))
        tp_pool = ctx.enter_context(
            tc.tile_pool(name="tpsum", bufs=1, space="PSUM"))
        gi_psum_pool = ctx.enter_context(
            tc.tile_pool(name="gipsum", bufs=2, space="PSUM"))
        dram_pool = ctx.enter_context(
            tc.tile_pool(name="dram", bufs=1, space="DRAM"))
        gst_pool = ctx.enter_context(tc.tile_pool(name="gst", bufs=3))
        gicp_pool = ctx.enter_context(tc.tile_pool(name="gicp", bufs=2))

        identr = pers.tile([128, B], BF16, tag="identr")
        nc.sync.dma_start(identr[:], inp["identr"][:])
        ident128 = pers.tile([128, 128], BF16, tag="ident128")
        nc.sync.dma_start(ident128[:], inp["ident128"][:])
        ones_bf = pers.tile([1, 128], BF16, tag="ones")
        nc.sync.dma_start(ones_bf[:], inp["ones_bf"][:])

        gi_dram = [dram_pool.tile([2, T, 4, B, 768], BF16, tag=f"gi{l}",
                                  name=f"gi_dram{l}") for l in range(3)]

        # ---------------- phase 0: BN stats ----------------
        # stat tile columns: 0 ms,1 ss,2 mu,3 ex2,4 mu2,5 ve,6 sqv,7 s0,
        # 8 s2,9 vs2,10 co,11 s,12 gam,13 bet,14 a,15 ma,16 negb
        stats = []
        with tc.tile_pool(name="ph0s", bufs=1) as ph0s:
            for si, p in ((0, 128), (1, 32)):
                st = ph0s.tile([p, 32], F32, tag=f"st{si}", name=f"st{si}")
                stats.append(st)
                xt = ph0s.tile([p, NTF], F32, tag=f"xt{si}", name=f"xt{si}")
                nc.sync.dma_start(xt[:], inp[f"xtf{si}"][:])
                C = lambda i: st[:, i:i+1]
                nc.vector.tensor_reduce(C(0), xt[:],
                                        axis=mybir.AxisListType.X, op=ALU.add)
                nc.scalar.activation(xt[:], xt[:], AF.Square, accum_out=C(1))
                nc.vector.tensor_scalar_mul(C(2), C(0), 1.0 / NTF)
                nc.vector.tensor_scalar_mul(C(3), C(1), 1.0 / NTF)
                nc.vector.tensor_mul(C(4), C(2), C(2))
                nc.vector.tensor_sub(C(5), C(3), C(4))
                nc.vector.tensor_scalar_add(C(5), C(5), EPS)
                nc.scalar.activation(C(6), C(5), AF.Sqrt)
                nc.vector.reciprocal(C(7), C(6))
                nc.vector.tensor_mul(C(8), C(7), C(7))
                nc.vector.tensor_mul(C(9), C(5), C(8))
                nc.vector.scalar_tensor_tensor(
                    C(10), C(9), -0.5, C(7), op0=ALU.mult, op1=ALU.mult)
                nc.vector.scalar_tensor_tensor(
                    C(11), C(7), 1.5, C(10), op0=ALU.mult, op1=ALU.add)
                nc.sync.dma_start(C(12), inp["gamma"][si*128: si*128+p, :])
                nc.sync.dma_start(C(13), inp["beta"][si*128: si*128+p, :])
                nc.vector.tensor_mul(C(14), C(12), C(11))
                nc.vector.tensor_mul(C(15), C(2), C(14))
                nc.vector.tensor_sub(C(16), C(13), C(15))

            # stat tiles stay alive: copy them into a pers tile
            stp = [pers.tile([p_, 32], F32, tag=f"stp{si_}", name=f"stp{si_}")
                   for si_, p_ in ((0, 128), (1, 32))]
            for si in range(2):
                nc.vector.tensor_copy(stp[si][:, 0:17], stats[si][:, 0:17])

        # ---------------- phase 0b: W0 fold + gi0 (two column halves) ----------------
        for nh in range(2):
            c0 = nh * 3072
            with tc.tile_pool(name=f"ph0w{nh}", bufs=1) as ph0w:
                w0a = ph0w.tile([128, 3072], F32, tag="w0a",
                                name=f"w0a{nh}")
                nc.sync.dma_start(w0a[:], inp["wih0a"][:, c0: c0 + 3072])
                w0b = ph0w.tile([32, 3072], F32, tag="w0b", name=f"w0b{nh}")
                nc.sync.dma_start(w0b[:], inp["wih0b"][:, c0: c0 + 3072])
                bias0 = ph0w.tile([1, 3072], F32, tag="bias0t",
                                  name=f"bias0t{nh}")
                nc.sync.dma_start(bias0[:], inp["bias0"][:, c0: c0 + 3072])

                w0rows = [(w0a, 128, stp[0]), (w0b, CIN - 128, stp[1])]
                for n in range(8):
                    bps = gi_psum_pool.tile([128, 384], F32, tag="gips",
                                            name=f"bps{nh}_{n}")
                    for ki, (w0, kp, st) in enumerate(w0rows):
                        nc.tensor.matmul(
                            bps[0:1, :], st[0:kp, 16:17],
                            w0[0:kp, n * 384: (n + 1) * 384],
                            start=(ki == 0), stop=(ki == 1))
                    nc.vector.tensor_add(bias0[:, n * 384: (n + 1) * 384],
                                         bps[0:1, :],
                                         bias0[:, n * 384: (n + 1) * 384])
                for w0, kp, st in w0rows:
                    nc.vector.tensor_scalar_mul(w0[0:kp, :], w0[0:kp, :],
                                                st[0:kp, 14:15])
                nc.sync.dma_start(w0b[CIN - 128: CIN - 128 + 1, :], bias0[:])
                w0ab = ph0w.tile([128, 3072], BF16, tag="w0ab",
                                 name=f"w0ab{nh}")
                w0bb = ph0w.tile([32, 3072], BF16, tag="w0bb",
                                 name=f"w0bb{nh}")
                nc.vector.tensor_copy(w0ab[:], w0a[:])
                nc.vector.tensor_copy(w0bb[0: CIN + 1 - 128, :],
                                      w0b[0: CIN + 1 - 128, :])

                xto_a = ph0w.tile([128, ROWS], BF16, tag="xtoa",
                                  name=f"xtoa{nh}")
                nc.sync.dma_start(xto_a[:], inp["xto"][0:128, :])
                xto_b = ph0w.tile([32, ROWS], BF16, tag="xtob",
                                  name=f"xtob{nh}")
                nc.sync.dma_start(xto_b[:], inp["xto"][128:160, :])

                for (r0, r1) in mchunks(0, ROWS):
                    mw = r1 - r0
                    for n in range(8):
                        gn = nh * 8 + n
                        gps = gi_psum_pool.tile([128, 384], F32, tag="gips",
                                                name=f"g0ps{nh}_{r0}_{n}")
                        nc.tensor.matmul(
                            gps[0:mw, :],
                            xto_a[:, r0:r1],
                            w0ab[:, n * 384: (n + 1) * 384],
                            start=True, stop=False)
                        nc.tensor.matmul(
                            gps[0:mw, :],
                            xto_b[0: CIN + 1 - 128, r0:r1],
                            w0bb[0: CIN + 1 - 128, n * 384: (n + 1) * 384],
                            start=False, stop=True)
                        gsb = gicp_pool.tile([128, 384], BF16, tag="gisb",
                                             name=f"g0sb{nh}_{r0}_{n}")
                        nc.vector.tensor_copy(gsb[0:mw, :], gps[0:mw, :])
                        d, g, half = gn // 8, (gn % 8) // 2, gn % 2
                        nc.sync.dma_start(
                            gi_dram[0][d, r0 // B: r1 // B, g, :,
                                       half * 384: (half + 1) * 384],
                            gsb[0:mw, :])

        # ---------------- layers ----------------
        scan_pool = ctx.enter_context(tc.tile_pool(name="scan", bufs=1))
        outT = None
        for l in range(3):
            whh_sb = scan_pool.tile([128, 2 * 8 * 3072], BF16, tag="whh_sb",
                               name=f"whh_sb{l}")
            for d in range(2):
                nc.sync.dma_start(whh_sb[:, d * 24576: (d + 1) * 24576],
                                  inp["whh"][l, d])
            bhhn_sb = scan_pool.tile([1, 2 * 1024], BF16, tag="bhhn_sb",
                                name=f"bhhn_sb{l}")
            for d in range(2):
                nc.sync.dma_start(bhhn_sb[:, d * 1024: (d + 1) * 1024],
                                  inp["bhhn"][l, d])

            outT = [scan_pool.tile([128, 8 * ROWS], BF16, tag=f"outT{d}",
                              name=f"outT{d}_{l}") for d in range(2)]
            zlhs = scan_pool.tile([128, B], BF16, tag="zlhs", name=f"zlhs{l}")
            nc.vector.memset(zlhs[:], 0.0)
            h_elem = [[scan_pool.tile([128, 256], BF16, tag=f"h{d}_{par}",
                                 name=f"h{d}_{par}_{l}")
                       for par in range(2)] for d in range(2)]
            for d in range(2):
                nc.vector.memset(h_elem[d][0][:], 0.0)

            # scratch layout (bf16 cols): rzs 0:512, rz 512:1024, t1 1024:1280,
            # t2 1280:1536, nt 1536:1792, dd 1792:2048, zd 2048:2304
            scr = [scan_pool.tile([128, 2304], BF16, tag=f"scr{d}",
                             name=f"scr{d}_{l}") for d in range(2)]

            for (ts, te) in windows:
                for t in range(ts, te):
                    gps_step = []
                    gst = gst_pool.tile([128, 1536], BF16, tag="gst",
                                        name=f"gst_{l}_{t}")
                    if DEBUG_INIT:
                        nc.vector.memset(gst[:], 0.0)
                    for d in range(2):
                        gp = gates_pool.tile([128, 768], F32, tag=f"gp{d}",
                                             name=f"gp{d}_{l}_{t}")
                        if DEBUG_INIT:
                            nc.vector.memset(gp[:], 0.0)
                        gps_step.append(gp)
                        wd0 = d * 24576
                        for g in range(4):
                            for (c0, cw) in ((0, 512), (512, 256)):
                                for k in range(8):
                                    if t == 0 or NO_TRANSPOSE:
                                        lhsT = zlhs[:, 0:B]
                                    else:
                                        lhsT = outT[d][:, k * ROWS + (t-1) * B:
                                                       k * ROWS + t * B]
                                    if k == 6 and c0 == 512:
                                        nc.tensor.matmul(
                                            gp[32*g: 32*g + B, 512:768],
                                            ones_bf[:, 0:B],
                                            bhhn_sb[:, d * 1024 + g * 256:
                                                    d * 1024 + (g+1) * 256],
                                            start=False, stop=False,
                                            tile_position=(0, 32 * g))
                                    nc.tensor.matmul(
                                        gp[32*g: 32*g + B, c0: c0 + cw],
                                        lhsT,
                                        whh_sb[:, wd0 + k * 3072 + g * 768
                                               + c0:
                                               wd0 + k * 3072 + g * 768
                                               + c0 + cw],
                                        start=(k == 0), stop=(k == 7),
                                        tile_position=(0, 32 * g))
                        for g in range(4):
                            nc.sync.dma_start(
                                gst[32*g: 32*g + B,
                                    d * 768: (d + 1) * 768],
                                gi_dram[l][d, t, g])

                    for d in range(2):
                        gp = gps_step[d]
                        gv = gst[:, d * 768: (d + 1) * 768]
                        h_prev = h_elem[d][t % 2]
                        h_new = h_elem[d][(t + 1) % 2]
                        sc = scr[d]
                        rzs, rz = sc[:, 0:512], sc[:, 512:1024]
                        t1, t2 = sc[:, 1024:1280], sc[:, 1280:1536]
                        nt, dd = sc[:, 1536:1792], sc[:, 1792:2048]
                        zd = sc[:, 2048:2304]
                        nc.vector.tensor_add(rzs, gp[:, 0:512], gv[:, 0:512])
                        nc.scalar.activation(rz, rzs, AF.Sigmoid)
                        nc.vector.tensor_mul(t1, rz[:, 0:256], gp[:, 512:768])
                        nc.vector.tensor_add(t2, t1, gv[:, 512:768])
                        nc.scalar.activation(nt, t2, AF.Tanh)
                        nc.vector.scalar_tensor_tensor(
                            dd, nt, -1.0, h_prev[:], op0=ALU.mult, op1=ALU.add)
                        nc.vector.tensor_mul(zd, rz[:, 256:512], dd)
                        nc.vector.tensor_add(h_new[:], zd, nt)

                        dst = outT[d].rearrange("p (k r) -> p k r", k=8)[
                            :, :, t * B: (t + 1) * B]
                        if NO_TRANSPOSE:
                            nc.vector.tensor_copy(
                                dst, h_new[:, 0:64].rearrange(
                                    "p (k b) -> p k b", k=8))
                            continue
                        tp = tp_pool.tile([128, 256], BF16, tag=f"tp{d}",
                                          name=f"tp{d}_{l}_{t}")
                        for j2 in range(2):
                            nc.tensor.transpose(
                                tp[:, j2 * 128: (j2 + 1) * 128],
                                h_new[:, j2 * 128: (j2 + 1) * 128],
                                ident128[:])
                        # tp col j2*128 + 32g + b holds h'[32g+b, 256g+128j2+p]
                        dst2 = outT[d].rearrange(
                            "p (g j2 r) -> p g j2 r", g=4, j2=2)[
                            :, :, :, t * B: (t + 1) * B]
                        src = tp.rearrange(
                            "p (j2 g b) -> p g j2 b", j2=2, g=4)[:, :, :, 0:B]
                        nc.vector.tensor_copy(dst2, src)

                if l < 2:
                    with tc.tile_pool(name=f"wih_l{l}_{ts}", bufs=2) as wp:
                        for n in range(16):
                            wt = wp.tile([128, 16 * 384], BF16, tag="wihT",
                                         name=f"wt_{l}_{ts}_{n}")
                            for k in range(16):
                                nc.sync.dma_start(
                                    wt[:, k * 384: (k + 1) * 384],
                                    inp["wih12"][l, k, :,
                                                 n * 384: (n + 1) * 384])
                            bt = wp.tile([1, 384], BF16, tag="biasT",
                                         name=f"bt_{l}_{ts}_{n}")
                            nc.sync.dma_start(
                                bt[:], inp["bias12"][l, :,
                                                     n * 384: (n + 1) * 384])
                            for (r0, r1) in mchunks(ts * B, te * B):
                                mw = r1 - r0
                                gps = gi_psum_pool.tile(
                                    [128, 384], F32, tag="gips",
                                    name=f"gps_{l}_{ts}_{n}_{r0}")
                                for k in range(16):
                                    dsrc, kk = k // 8, k % 8
                                    nc.tensor.matmul(
                                        gps[0:mw, :],
                                        outT[dsrc][:, kk * ROWS + r0:
                                                   kk * ROWS + r1],
                                        wt[:, k * 384: (k + 1) * 384],
                                        start=(k == 0), stop=False)
                                nc.tensor.matmul(
                                    gps[0:mw, :], ones_bf[:, 0:mw], bt[:],
                                    start=False, stop=True)
                                gsb = gicp_pool.tile(
                                    [128, 384], BF16, tag="gisb",
                                    name=f"gsb_{l}_{ts}_{n}_{r0}")
                                nc.vector.tensor_copy(gsb[0:mw, :],
                                                      gps[0:mw, :])
                                d, g, half = n // 8, (n % 8) // 2, n % 2
                                nc.sync.dma_start(
                                    gi_dram[l + 1][
                                        d, r0 // B: r1 // B, g, :,
                                        half * 384: (half + 1) * 384],
                                    gsb[0:mw, :])

        # ---------------- FC ----------------
        fcw = pers.tile([128, 16 * NCLS], BF16, tag="fcw")
        for k in range(16):
            nc.sync.dma_start(fcw[:, k * NCLS: (k + 1) * NCLS],
                              inp["fcwT"][k])
        fcb = pers.tile([1, NCLS], BF16, tag="fcb")
        nc.sync.dma_start(fcb[:], inp["fcb"][:])
        fps = gi_psum_pool.tile([128, 384], F32, tag="gips", name="fps")
        for k in range(16):
            d, kk = k // 8, k % 8
            nc.tensor.matmul(
                fps[0:B, 0:NCLS],
                outT[d][:, kk * ROWS + (T - 1) * B: kk * ROWS + T * B],
                fcw[:, k * NCLS: (k + 1) * NCLS],
                start=(k == 0), stop=False)
        nc.tensor.matmul(fps[0:B, 0:NCLS], ones_bf[:, 0:B], fcb[:],
                         start=False, stop=True)
        fout = gicp_pool.tile([B, NCLS], F32, tag="fout")
        nc.vector.tensor_copy(fout[:], fps[0:B, 0:NCLS])
        nc.sync.dma_start(out_t[:], fout[:])

        ctx.close()

    nc.compile()
    return nc


_cache = {}


def kernel(**inputs):
    T = inputs["x"].shape[1]
    n_full = inputs["x"].shape[0]
    key = ("prog", T, n_full)
    if key not in _cache:
        _cache[key] = build_program(T, n_full)
    nc = _cache[key]
    per_core = host_prep(inputs, T, n_full)
    res = run_bass_kernel_spmd(nc, per_core, core_ids=list(range(NCORES)))
    out = np.concatenate([r["out"] for r in res.results], axis=0)
    return np.ascontiguousarray(out.astype(np.float32))


# revision 17
# speedup vs baseline: 1.5330x; 1.5330x over previous
"""Trainium2 Bass kernel for a 3-layer bidirectional GRU classifier.

Model (faithful to the reference):
  BatchNorm1d (train-mode batch stats over (N,T)) -> 3 stacked biGRU layers
  (both "directions" scan forward in time, per the reference's quirk) ->
  last-timestep hidden concat -> Linear to 60 classes.

Strategy: pure data parallelism over batch (8 samples per core, 8 cores, no
collectives).  Per core the recurrent gate matmul streams W_hh^T through the
PE array with the per-sample hidden states h^T as the stationary operand,
using 4-way column tiling (tile_position col groups) so four weight-quarter
streams run concurrently.  The two directions ping-pong so DVE/ACT gate math
of one direction overlaps PE streaming of the other.  Input projections (gi)
for layer l+1 are computed as large matmuls from the transposed layer-l
outputs (built incrementally by per-step PE transposes) and staged in DRAM.
BatchNorm is folded into the layer-0 input projection on device.
"""

import os
import sys

for _p in ("/opt/trn_rl_repo", "/root/.axon_site/_ro/trn_rl_repo"):
    if os.path.isdir(_p) and _p not in sys.path:
        sys.path.append(_p)

import numpy as np
import ml_dtypes

import concourse.bacc as bacc
import concourse.mybir as mybir
from concourse.tile import TileContext
from concourse.bass_utils import run_bass_kernel_spmd

BF16 = mybir.dt.bfloat16
F32 = mybir.dt.float32
F32R = mybir.dt.float32r
AF = mybir.ActivationFunctionType
ALU = mybir.AluOpType

NCORES = 8
DEBUG_INIT = False  # memset garbage regions so CoreSim's uninit checks pass
NO_TRANSPOSE = False  # bisect flag: skip PE transposes (breaks correctness)
SIM_SAFE_ORDER = False  # serial col-groups (slow, but CoreSim value-exact)
H = 1024
G3 = 3 * H
CIN = 150
NCLS = 60
EPS = 1e-5


def _perm3h(v):
    """Reorder a [..., 3072] gate-major vector to (g, a, cj) column order."""
    a = v.reshape(*v.shape[:-1], 3, 4, 256)
    a = np.moveaxis(a, -3, -2)  # [..., 4, 3, 256]
    return a.reshape(*v.shape[:-1], G3)


def host_prep(inputs, T, n_full):
    B = n_full // NCORES
    x = np.asarray(inputs["x"], np.float32)
    gamma = np.asarray(inputs["bn_gamma"], np.float32)
    beta = np.asarray(inputs["bn_beta"], np.float32)
    w_ih0 = np.asarray(inputs["w_ih0"], np.float32)
    w_hh0 = np.asarray(inputs["w_hh0"], np.float32)
    b_ih0 = np.asarray(inputs["b_ih0"], np.float32)
    b_hh0 = np.asarray(inputs["b_hh0"], np.float32)
    w_ih = np.asarray(inputs["w_ih"], np.float32)
    w_hh = np.asarray(inputs["w_hh"], np.float32)
    b_ih = np.asarray(inputs["b_ih"], np.float32)
    b_hh = np.asarray(inputs["b_hh"], np.float32)
    fc_w = np.asarray(inputs["fc_w"], np.float32)
    fc_b = np.asarray(inputs["fc_b"], np.float32)

    shared = {}
    NTF = T * n_full

    xTf = np.ascontiguousarray(x.transpose(2, 1, 0).reshape(CIN, NTF))
    shared["xtf0"] = np.ascontiguousarray(xTf[:128])
    xtf1 = np.zeros((32, NTF), np.float32)
    xtf1[: CIN - 128] = xTf[128:]
    shared["xtf1"] = xtf1

    # W_hh^T permuted bf16: [3, 2, 128, 8*3072], free = k*3072+g*768+a*256+cj
    whh_all = np.stack([w_hh0, w_hh[0], w_hh[1]])  # [3,2,3072,1024]
    t = whh_all.reshape(3, 2, 3, 4, 256, 8, 128)
    t = np.transpose(t, (0, 1, 6, 5, 3, 2, 4))
    shared["whh"] = np.ascontiguousarray(
        t.reshape(3, 2, 128, 8 * 3072).astype(ml_dtypes.bfloat16))

    # W_ih layers 1,2: [2, 16, 128, 6144], col = d*3072+g*768+a*256+cj
    t = w_ih.reshape(2, 2, 3, 4, 256, 16, 128)
    t = np.transpose(t, (0, 5, 6, 1, 3, 2, 4))
    shared["wih12"] = np.ascontiguousarray(
        t.reshape(2, 16, 128, 6144).astype(ml_dtypes.bfloat16))

    # W_ih0^T permuted fp32 [150->(128,32), 6144]
    t = w_ih0.reshape(2, 3, 4, 256, CIN)
    t = np.transpose(t, (4, 0, 2, 1, 3)).reshape(CIN, 6144)
    shared["wih0a"] = np.ascontiguousarray(t[:128])
    w0b = np.zeros((32, 6144), np.float32)
    w0b[: CIN - 128] = t[128:]
    shared["wih0b"] = w0b

    mask_rz = np.zeros(G3, np.float32)
    mask_rz[: 2 * H] = 1.0
    bias0 = np.concatenate(
        [_perm3h(b_ih0[d] + b_hh0[d] * mask_rz) for d in range(2)])
    shared["bias0"] = bias0.reshape(1, 6144)

    b12 = [np.concatenate(
        [_perm3h(b_ih[li, d] + b_hh[li, d] * mask_rz) for d in range(2)])
        for li in range(2)]
    shared["bias12"] = np.stack(b12).reshape(2, 1, 6144).astype(
        ml_dtypes.bfloat16)

    bhh_all = np.stack([b_hh0, b_hh[0], b_hh[1]])[:, :, 2 * H:]  # [3,2,1024]
    shared["bhhn"] = np.ascontiguousarray(
        bhh_all.reshape(3, 2, 1, 1024).astype(ml_dtypes.bfloat16))

    shared["fcwT"] = np.ascontiguousarray(
        fc_w.T.reshape(16, 128, NCLS).astype(ml_dtypes.bfloat16))
    shared["fcb"] = fc_b.reshape(1, NCLS).astype(ml_dtypes.bfloat16)

    shared["gamma"] = np.concatenate(
        [gamma, np.zeros(10, np.float32)]).reshape(160, 1)
    shared["beta"] = np.concatenate(
        [beta, np.zeros(10, np.float32)]).reshape(160, 1)

    shared["ones_bf"] = np.ones((1, 128), ml_dtypes.bfloat16)
    # identity rows replicated at each 32-partition block (for col-group
    # local transposes): row p holds e_{p%32} when p%32 < B else 0
    idr = np.zeros((128, B), ml_dtypes.bfloat16)
    for p in range(128):
        if p % 32 < B:
            idr[p, p % 32] = 1
    shared["identr"] = idr
    shared["ident128"] = np.eye(128, dtype=ml_dtypes.bfloat16)

    per_core = []
    for c in range(NCORES):
        m = dict(shared)
        xo = x[c * B: (c + 1) * B]
        xT = xo.transpose(2, 1, 0).reshape(CIN, T * B)
        aug = np.zeros((160, T * B), np.float32)
        aug[:CIN] = xT
        aug[CIN] = 1.0
        m["xto"] = aug.astype(ml_dtypes.bfloat16)
        per_core.append(m)
    return per_core


def build_program(T, n_full):
    B = n_full // NCORES
    ROWS = T * B
    NTF = T * n_full
    nc = bacc.Bacc("TRN2", target_bir_lowering=False, debug=False,
                   num_devices=NCORES)

    inp = {}
    def din(name, shape, dt):
        inp[name] = nc.dram_tensor(name, list(shape), dt, kind="ExternalInput")

    din("xtf0", (128, NTF), F32)
    din("xtf1", (32, NTF), F32)
    din("xto", (160, ROWS), BF16)
    din("whh", (3, 2, 128, 8 * 3072), BF16)
    din("wih12", (2, 16, 128, 6144), BF16)
    din("wih0a", (128, 6144), F32)
    din("wih0b", (32, 6144), F32)
    din("bias0", (1, 6144), F32)
    din("bias12", (2, 1, 6144), BF16)
    din("bhhn", (3, 2, 1, 1024), BF16)
    din("fcwT", (16, 128, NCLS), BF16)
    din("fcb", (1, NCLS), BF16)
    din("gamma", (160, 1), F32)
    din("beta", (160, 1), F32)
    din("ones_bf", (1, 128), BF16)
    din("identr", (128, B), BF16)
    din("ident128", (128, 128), BF16)

    out_t = nc.dram_tensor("out", [B, NCLS], F32, kind="ExternalOutput")

    WSTEPS = 64
    windows = []
    t0 = 0
    while t0 < T:
        windows.append((t0, min(t0 + WSTEPS, T)))
        t0 = min(t0 + WSTEPS, T)

    def mchunks(r0, r1):
        out = []
        while r0 < r1:
            out.append((r0, min(r0 + 128, r1)))
            r0 = min(r0 + 128, r1)
        return out

    with TileContext(nc) as tc:
        from contextlib import ExitStack
        ctx = ExitStack()
        pers = ctx.enter_context(tc.tile_pool(name="pers", bufs=1))
        gates_pool = ctx.enter_context(
            tc.tile_pool(name="gates", bufs=1, space="PSUM"))
        tp_pool = ctx.enter_context(
            tc.tile_pool(name="tpsum", bufs=1, space="PSUM"))
        gi_psum_pool = ctx.enter_context(
            tc.tile_pool(name="gipsum", bufs=2, space="PSUM"))
        dram_pool = ctx.enter_context(
            tc.tile_pool(name="dram", bufs=1, space="DRAM"))
        gst_pool = ctx.enter_context(tc.tile_pool(name="gst", bufs=3))
        gicp_pool = ctx.enter_context(tc.tile_pool(name="gicp", bufs=2))

        identr = pers.tile([128, B], BF16, tag="identr")
        nc.sync.dma_start(identr[:], inp["identr"][:])
        ident128 = pers.tile([128, 128], BF16, tag="ident128")
        nc.sync.dma_start(ident128[:], inp["ident128"][:])
        ones_bf = pers.tile([1, 128], BF16, tag="ones")
        nc.sync.dma_start(ones_bf[:], inp["ones_bf"][:])

        gi_dram = [dram_pool.tile([2, T, 4, B, 768], BF16, tag=f"gi{l}",
                                  name=f"gi_dram{l}") for l in range(3)]

        # ---------------- phase 0: BN stats ----------------
        # stat tile columns: 0 ms,1 ss,2 mu,3 ex2,4 mu2,5 ve,6 sqv,7 s0,
        # 8 s2,9 vs2,10 co,11 s,12 gam,13 bet,14 a,15 ma,16 negb
        stats = []
        with tc.tile_pool(name="ph0s", bufs=1) as ph0s:
            for si, p in ((0, 128), (1, 32)):
                st = ph0s.tile([p, 32], F32, tag=f"st{si}", name=f"st{si}")
                stats.append(st)
                xt = ph0s.tile([p, NTF], F32, tag=f"xt{si}", name=f"xt{si}")
                nc.sync.dma_start(xt[:], inp[f"xtf{si}"][:])
                C = lambda i: st[:, i:i+1]
                nc.vector.tensor_reduce(C(0), xt[:],
                                        axis=mybir.AxisListType.X, op=ALU.add)
                nc.scalar.activation(xt[:], xt[:], AF.Square, accum_out=C(1))
                nc.vector.tensor_scalar_mul(C(2), C(0), 1.0 / NTF)
                nc.vector.tensor_scalar_mul(C(3), C(1), 1.0 / NTF)
                nc.vector.tensor_mul(C(4), C(2), C(2))
                nc.vector.tensor_sub(C(5), C(3), C(4))
                nc.vector.tensor_scalar_add(C(5), C(5), EPS)
                nc.scalar.activation(C(6), C(5), AF.Sqrt)
                nc.vector.reciprocal(C(7), C(6))
                nc.vector.tensor_mul(C(8), C(7), C(7))
                nc.vector.tensor_mul(C(9), C(5), C(8))
                nc.vector.scalar_tensor_tensor(
                    C(10), C(9), -0.5, C(7), op0=ALU.mult, op1=ALU.mult)
                nc.vector.scalar_tensor_tensor(
                    C(11), C(7), 1.5, C(10), op0=ALU.mult, op1=ALU.add)
                nc.sync.dma_start(C(12), inp["gamma"][si*128: si*128+p, :])
                nc.sync.dma_start(C(13), inp["beta"][si*128: si*128+p, :])
                nc.vector.tensor_mul(C(14), C(12), C(11))
                nc.vector.tensor_mul(C(15), C(2), C(14))
                nc.vector.tensor_sub(C(16), C(13), C(15))

            # stat tiles stay alive: copy them into a pers tile
            stp = [pers.tile([p_, 32], F32, tag=f"stp{si_}", name=f"stp{si_}")
                   for si_, p_ in ((0, 128), (1, 32))]
            for si in range(2):
                nc.vector.tensor_copy(stp[si][:, 0:17], stats[si][:, 0:17])

        # ---------------- phase 0b: W0 fold + gi0 (two column halves) ----------------
        for nh in range(2):
            c0 = nh * 3072
            with tc.tile_pool(name=f"ph0w{nh}", bufs=1) as ph0w:
                w0a = ph0w.tile([128, 3072], F32, tag="w0a",
                                name=f"w0a{nh}")
                nc.sync.dma_start(w0a[:], inp["wih0a"][:, c0: c0 + 3072])
                w0b = ph0w.tile([32, 3072], F32, tag="w0b", name=f"w0b{nh}")
                nc.sync.dma_start(w0b[:], inp["wih0b"][:, c0: c0 + 3072])
                bias0 = ph0w.tile([1, 3072], F32, tag="bias0t",
                                  name=f"bias0t{nh}")
                nc.sync.dma_start(bias0[:], inp["bias0"][:, c0: c0 + 3072])

                w0rows = [(w0a, 128, stp[0]), (w0b, CIN - 128, stp[1])]
                for n in range(8):
                    bps = gi_psum_pool.tile([128, 384], F32, tag="gips",
                                            name=f"bps{nh}_{n}")
                    for ki, (w0, kp, st) in enumerate(w0rows):
                        nc.tensor.matmul(
                            bps[0:1, :], st[0:kp, 16:17],
                            w0[0:kp, n * 384: (n + 1) * 384],
                            start=(ki == 0), stop=(ki == 1))
                    nc.vector.tensor_add(bias0[:, n * 384: (n + 1) * 384],
                                         bps[0:1, :],
                                         bias0[:, n * 384: (n + 1) * 384])
                for w0, kp, st in w0rows:
                    nc.vector.tensor_scalar_mul(w0[0:kp, :], w0[0:kp, :],
                                                st[0:kp, 14:15])
                nc.sync.dma_start(w0b[CIN - 128: CIN - 128 + 1, :], bias0[:])
                w0ab = ph0w.tile([128, 3072], BF16, tag="w0ab",
                                 name=f"w0ab{nh}")
                w0bb = ph0w.tile([32, 3072], BF16, tag="w0bb",
                                 name=f"w0bb{nh}")
                nc.vector.tensor_copy(w0ab[:], w0a[:])
                nc.vector.tensor_copy(w0bb[0: CIN + 1 - 128, :],
                                      w0b[0: CIN + 1 - 128, :])

                xto_a = ph0w.tile([128, ROWS], BF16, tag="xtoa",
                                  name=f"xtoa{nh}")
                nc.sync.dma_start(xto_a[:], inp["xto"][0:128, :])
                xto_b = ph0w.tile([32, ROWS], BF16, tag="xtob",
                                  name=f"xtob{nh}")
                nc.sync.dma_start(xto_b[:], inp["xto"][128:160, :])

                for (r0, r1) in mchunks(0, ROWS):
                    mw = r1 - r0
                    for n in range(8):
                        gn = nh * 8 + n
                        gps = gi_psum_pool.tile([128, 384], F32, tag="gips",
                                                name=f"g0ps{nh}_{r0}_{n}")
                        nc.tensor.matmul(
                            gps[0:mw, :],
                            xto_a[:, r0:r1],
                            w0ab[:, n * 384: (n + 1) * 384],
                            start=True, stop=False)
                        nc.tensor.matmul(
                            gps[0:mw, :],
                            xto_b[0: CIN + 1 - 128, r0:r1],
                            w0bb[0: CIN + 1 - 128, n * 384: (n + 1) * 384],
                            start=False, stop=True)
                        gsb = gicp_pool.tile([128, 384], BF16, tag="gisb",
                                             name=f"g0sb{nh}_{r0}_{n}")
                        nc.vector.tensor_copy(gsb[0:mw, :], gps[0:mw, :])
                        d, g, half = gn // 8, (gn % 8) // 2, gn % 2
                        nc.sync.dma_start(
                            gi_dram[0][d, r0 // B: r1 // B, g, :,
                                       half * 384: (half + 1) * 384],
                            gsb[0:mw, :])

        # ---------------- layers ----------------
        scan_pool = ctx.enter_context(tc.tile_pool(name="scan", bufs=1))
        outT = None
        for l in range(3):
            whh_sb = scan_pool.tile([128, 2 * 8 * 3072], BF16, tag="whh_sb",
                               name=f"whh_sb{l}")
            for d in range(2):
                nc.sync.dma_start(whh_sb[:, d * 24576: (d + 1) * 24576],
                                  inp["whh"][l, d])
            bhhn_sb = scan_pool.tile([1, 2 * 1024], BF16, tag="bhhn_sb",
                                name=f"bhhn_sb{l}")
            for d in range(2):
                nc.sync.dma_start(bhhn_sb[:, d * 1024: (d + 1) * 1024],
                                  inp["bhhn"][l, d])

            outT = [scan_pool.tile([128, 8 * ROWS], BF16, tag=f"outT{d}",
                              name=f"outT{d}_{l}") for d in range(2)]
            zlhs = scan_pool.tile([128, B], BF16, tag="zlhs", name=f"zlhs{l}")
            nc.vector.memset(zlhs[:], 0.0)
            h_elem = [[scan_pool.tile([128, 256], BF16, tag=f"h{d}_{par}",
                                 name=f"h{d}_{par}_{l}")
                       for par in range(2)] for d in range(2)]
            for d in range(2):
                nc.vector.memset(h_elem[d][0][:], 0.0)

            # scratch layout (bf16 cols): rzs 0:512, rz 512:1024, t1 1024:1280,
            # t2 1280:1536, nt 1536:1792, dd 1792:2048, zd 2048:2304
            scr = [scan_pool.tile([128, 2304], BF16, tag=f"scr{d}",
                             name=f"scr{d}_{l}") for d in range(2)]

            for (ts, te) in windows:
                for t in range(ts, te):
                    gps_step = []
                    gst = gst_pool.tile([128, 1536], BF16, tag="gst",
                                        name=f"gst_{l}_{t}")
                    if DEBUG_INIT:
                        nc.vector.memset(gst[:], 0.0)
                    for d in range(2):
                        gp = gates_pool.tile([128, 768], F32, tag=f"gp{d}",
                                             name=f"gp{d}_{l}_{t}")
                        if DEBUG_INIT:
                            nc.vector.memset(gp[:], 0.0)
                        gps_step.append(gp)
                        wd0 = d * 24576
                        if SIM_SAFE_ORDER:
                            mm_order = [(g, c0, cw, k) for g in range(4)
                                        for (c0, cw) in ((0, 512), (512, 256))
                                        for k in range(8)]
                        else:
                            mm_order = [(g, c0, cw, k)
                                        for (c0, cw) in ((0, 512), (512, 256))
                                        for k in range(8) for g in range(4)]
                        sgc = not SIM_SAFE_ORDER
                        for (g, c0, cw, k) in mm_order:
                            if t == 0 or NO_TRANSPOSE:
                                lhsT = zlhs[:, 0:B]
                            else:
                                lhsT = outT[d][:, k * ROWS + (t-1) * B:
                                               k * ROWS + t * B]
                            if k == 6 and c0 == 512:
                                nc.tensor.matmul(
                                    gp[32*g: 32*g + B, 512:768],
                                    ones_bf[:, 0:B],
                                    bhhn_sb[:, d * 1024 + g * 256:
                                            d * 1024 + (g+1) * 256],
                                    start=False, stop=False,
                                    skip_group_check=sgc,
                                    tile_position=(0, 32 * g))
                            nc.tensor.matmul(
                                gp[32*g: 32*g + B, c0: c0 + cw],
                                lhsT,
                                whh_sb[:, wd0 + k * 3072 + g * 768 + c0:
                                       wd0 + k * 3072 + g * 768 + c0 + cw],
                                start=(k == 0), stop=(k == 7),
                                skip_group_check=sgc,
                                tile_position=(0, 32 * g))
                        for g in range(4):
                            nc.sync.dma_start(
                                gst[32*g: 32*g + B,
                                    d * 768: (d + 1) * 768],
                                gi_dram[l][d, t, g])

                    for d in range(2):
                        gp = gps_step[d]
                        gv = gst[:, d * 768: (d + 1) * 768]
                        h_prev = h_elem[d][t % 2]
                        h_new = h_elem[d][(t + 1) % 2]
                        sc = scr[d]
                        rzs, rz = sc[:, 0:512], sc[:, 512:1024]
                        t1, t2 = sc[:, 1024:1280], sc[:, 1280:1536]
                        nt, dd = sc[:, 1536:1792], sc[:, 1792:2048]
                        zd = sc[:, 2048:2304]
                        nc.vector.tensor_add(rzs, gp[:, 0:512], gv[:, 0:512])
                        nc.scalar.activation(rz, rzs, AF.Sigmoid)
                        nc.vector.tensor_mul(t1, rz[:, 0:256], gp[:, 512:768])
                        nc.vector.tensor_add(t2, t1, gv[:, 512:768])
                        nc.scalar.activation(nt, t2, AF.Tanh)
                        nc.vector.scalar_tensor_tensor(
                            dd, nt, -1.0, h_prev[:], op0=ALU.mult, op1=ALU.add)
                        nc.vector.tensor_mul(zd, rz[:, 256:512], dd)
                        nc.vector.tensor_add(h_new[:], zd, nt)

                        dst = outT[d].rearrange("p (k r) -> p k r", k=8)[
                            :, :, t * B: (t + 1) * B]
                        if NO_TRANSPOSE:
                            nc.vector.tensor_copy(
                                dst, h_new[:, 0:64].rearrange(
                                    "p (k b) -> p k b", k=8))
                            continue
                        tp = tp_pool.tile([128, 256], BF16, tag=f"tp{d}",
                                          name=f"tp{d}_{l}_{t}")
                        for j2 in range(2):
                            nc.tensor.transpose(
                                tp[:, j2 * 128: (j2 + 1) * 128],
                                h_new[:, j2 * 128: (j2 + 1) * 128],
                                ident128[:])
                        # tp col j2*128 + 32g + b holds h'[32g+b, 256g+128j2+p]
                        dst2 = outT[d].rearrange(
                            "p (g j2 r) -> p g j2 r", g=4, j2=2)[
                            :, :, :, t * B: (t + 1) * B]
                        src = tp.rearrange(
                            "p (j2 g b) -> p g j2 b", j2=2, g=4)[:, :, :, 0:B]
                        nc.vector.tensor_copy(dst2, src)

                if l < 2:
                    with tc.tile_pool(name=f"wih_l{l}_{ts}", bufs=2) as wp:
                        for n in range(16):
                            wt = wp.tile([128, 16 * 384], BF16, tag="wihT",
                                         name=f"wt_{l}_{ts}_{n}")
                            for k in range(16):
                                nc.sync.dma_start(
                                    wt[:, k * 384: (k + 1) * 384],
                                    inp["wih12"][l, k, :,
                                                 n * 384: (n + 1) * 384])
                            bt = wp.tile([1, 384], BF16, tag="biasT",
                                         name=f"bt_{l}_{ts}_{n}")
                            nc.sync.dma_start(
                                bt[:], inp["bias12"][l, :,
                                                     n * 384: (n + 1) * 384])
                            for (r0, r1) in mchunks(ts * B, te * B):
                                mw = r1 - r0
                                gps = gi_psum_pool.tile(
                                    [128, 384], F32, tag="gips",
                                    name=f"gps_{l}_{ts}_{n}_{r0}")
                                for k in range(16):
                                    dsrc, kk = k // 8, k % 8
                                    nc.tensor.matmul(
                                        gps[0:mw, :],
                                        outT[dsrc][:, kk * ROWS + r0:
                                                   kk * ROWS + r1],
                                        wt[:, k * 384: (k + 1) * 384],
                                        start=(k == 0), stop=False)
                                nc.tensor.matmul(
                                    gps[0:mw, :], ones_bf[:, 0:mw], bt[:],
                                    start=False, stop=True)
                                gsb = gicp_pool.tile(
                                    [128, 384], BF16, tag="gisb",
                                    name=f"gsb_{l}_{ts}_{n}_{r0}")
                                nc.vector.tensor_copy(gsb[0:mw, :],
                                                      gps[0:mw, :])
                                d, g, half = n // 8, (n % 8) // 2, n % 2
                                nc.sync.dma_start(
                                    gi_dram[l + 1][
                                        d, r0 // B: r1 // B, g, :,
                                        half * 384: (half + 1) * 384],
                                    gsb[0:mw, :])

        # ---------------- FC ----------------
        fcw = pers.tile([128, 16 * NCLS], BF16, tag="fcw")
        for k in range(16):
            nc.sync.dma_start(fcw[:, k * NCLS: (k + 1) * NCLS],
                              inp["fcwT"][k])
        fcb = pers.tile([1, NCLS], BF16, tag="fcb")
        nc.sync.dma_start(fcb[:], inp["fcb"][:])
        fps = gi_psum_pool.tile([128, 384], F32, tag="gips", name="fps")
        for k in range(16):
            d, kk = k // 8, k % 8
            nc.tensor.matmul(
                fps[0:B, 0:NCLS],
                outT[d][:, kk * ROWS + (T - 1) * B: kk * ROWS + T * B],
                fcw[:, k * NCLS: (k + 1) * NCLS],
                start=(k == 0), stop=False)
        nc.tensor.matmul(fps[0:B, 0:NCLS], ones_bf[:, 0:B], fcb[:],
                         start=False, stop=True)
        fout = gicp_pool.tile([B, NCLS], F32, tag="fout")
        nc.vector.tensor_copy(fout[:], fps[0:B, 0:NCLS])
        nc.sync.dma_start(out_t[:], fout[:])

        ctx.close()

    nc.compile()
    return nc


_cache = {}


def kernel(**inputs):
    T = inputs["x"].shape[1]
    n_full = inputs["x"].shape[0]
    key = ("prog", T, n_full)
    if key not in _cache:
        _cache[key] = build_program(T, n_full)
    nc = _cache[key]
    per_core = host_prep(inputs, T, n_full)
    res = run_bass_kernel_spmd(nc, per_core, core_ids=list(range(NCORES)))
    out = np.concatenate([r["out"] for r in res.results], axis=0)
    return np.ascontiguousarray(out.astype(np.float32))
